# revision 9
# baseline (speedup 1.0000x reference)
"""StyleGAN2 up-2x blur (upfirdn2d, up=2, pad=(2,1), 4x4 kernel) on 8 trn2 cores.

x: (4, 64, 256, 256) f32, kernel: (4, 4) f32 -> out: (4, 64, 511, 511) f32.

Polyphase decomposition: out[2r+s, 2c+t] is a 2x2-tap conv of x with weights
from the flipped kernel w = kernel[::-1, ::-1]:
  s=0 -> vertical taps (w[0,kx] @ r-1, w[2,kx] @ r); s=1 -> (w[1,kx] @ r, w[3,kx] @ r+1)
  t=0 -> horizontal taps kx in {0 (c-1), 2 (c)};    t=1 -> kx in {1 (c), 3 (c+1)}

Sharding: pure data parallel over the 256 (N*C) planes, 32 planes/core.

Device algorithm (per core): the vertical 2-tap combine runs on TensorE as
banded-matrix matmuls (stationary [128,128] bands), with the horizontal taps
folded in as column-shifted moving operands accumulating into the same PSUM
bank.  fp32 inputs are split host-side into bf16 hi+lo (x = hi + lo, packed
into one [P,256,2,256] array so loads move 1KB-contiguous chunks); each
logical tap is 2 bf16 matmuls (1 cy/row) with fp32 PSUM accumulation; total
error ~2^-18 rel.  Two planes are packed per matmul (moving free = 512).

Output rows are assembled as row-PAIRS per partition ([127, 2, 511] tiles:
partition i holds rows 2i/2i+1 of a 254-row block), so every store DMA is one
fully contiguous 519KB HBM range with 4088B per-partition chunks.  Rows
254/255/256 (the chunk seam) are computed separately via diagonal matmuls
over plane-partitioned single-row tiles.  Stores go through SWDGE
(nc.gpsimd) which spreads packets over all 16 SDMA engines; HWDGE stores
were measured 10x slower.
"""

import os
import numpy as np
import ml_dtypes

_BF = ml_dtypes.bfloat16
_NCORES = 8
_PL = 32            # planes per core
_H = 256
_W = 256
_OW = 511

_cache = {}
last_exec_ns = None
last_results = None


def _build(wlo_nz: bool):
    from contextlib import ExitStack
    import concourse.mybir as mybir
    import concourse.tile as tile
    from concourse import bacc

    BF = mybir.dt.bfloat16
    F32 = mybir.dt.float32

    nc = bacc.Bacc("TRN2", target_bir_lowering=False, debug=False)
    xp = nc.dram_tensor("xp", [_PL, _H, 2, _W], BF, kind="ExternalInput").ap()
    sth = nc.dram_tensor("sth", [128, 12, 128], BF, kind="ExternalInput").ap()
    dgh = nc.dram_tensor("dgh", [32, 16, 32], BF, kind="ExternalInput").ap()
    if wlo_nz:
        stl = nc.dram_tensor("stl", [128, 12, 128], BF, kind="ExternalInput").ap()
        dgl = nc.dram_tensor("dgl", [32, 16, 32], BF, kind="ExternalInput").ap()
    out = nc.dram_tensor("out", [_PL, _OW, _OW], F32, kind="ExternalOutput").ap()

    ncopy = 0  # alternate evacuation copies between VectorE and ScalarE

    with tile.TileContext(nc) as tc, ExitStack() as ctx:
        cpool = ctx.enter_context(tc.tile_pool(name="const", bufs=1))
        tpool = ctx.enter_context(tc.tile_pool(name="tin", bufs=8))
        epool = ctx.enter_context(tc.tile_pool(name="edge", bufs=1))
        opool = ctx.enter_context(tc.tile_pool(name="oasm", bufs=18))
        bpool = ctx.enter_context(tc.tile_pool(name="bnd", bufs=1))
        ppool = ctx.enter_context(tc.tile_pool(name="ps", bufs=8, space="PSUM"))

        sth_t = cpool.tile([128, 12, 128], BF)
        nc.sync.dma_start(out=sth_t[:, :, :], in_=sth)
        dgh_t = cpool.tile([32, 16, 32], BF)
        nc.sync.dma_start(out=dgh_t[:, :, :], in_=dgh)
        if wlo_nz:
            stl_t = cpool.tile([128, 12, 128], BF)
            nc.sync.dma_start(out=stl_t[:, :, :], in_=stl)
            dgl_t = cpool.tile([32, 16, 32], BF)
            nc.sync.dma_start(out=dgl_t[:, :, :], in_=dgl)

        def copy_out(dst, src):
            nonlocal ncopy
            if ncopy % 2 == 0:
                nc.vector.tensor_copy(out=dst, in_=src)
            else:
                nc.scalar.copy(out=dst, in_=src)
            ncopy += 1

        # ---- seam rows oy=254 (s0,r=127: x[126],x[127]), oy=255 (s1,r=127:
        # ---- x[127],x[128]), oy=256 (s0,r=128: x[127],x[128])
        etiles = {}
        for row in (126, 127, 128):
            e = epool.tile([32, 2, 256], BF, tag=f"e{row}")
            nc.sync.dma_start(out=e[:, :, :].rearrange("g a w -> g (a w)"),
                              in_=xp[:, row, :, :].rearrange("g a w -> g (a w)"))
            etiles[row] = e

        bt = bpool.tile([32, 3, _OW], F32)
        seams = ((0, ((0, 126), (2, 127))),   # oy254: ky=0 on x126, ky=2 on x127
                 (1, ((1, 127), (3, 128))),   # oy255
                 (2, ((0, 127), (2, 128))))   # oy256
        for bi, taps in seams:
            pb = ppool.tile([32, 2, 256], F32, tag="ps")
            mms = []
            # (t_, kx, moving col slice, psum col slice)
            for t_, kx, mv, pc in ((0, 2, (0, 256), (0, 256)), (0, 0, (0, 255), (1, 256)),
                                   (1, 1, (0, 255), (0, 255)), (1, 3, (1, 256), (0, 255))):
                for ky, erow in taps:
                    mms.append((t_, ky * 4 + kx, erow, 0, mv, pc, "h"))
                    mms.append((t_, ky * 4 + kx, erow, 1, mv, pc, "h"))
                    if wlo_nz:
                        mms.append((t_, ky * 4 + kx, erow, 0, mv, pc, "l"))
            for i, (t_, j, erow, hl, mv, pc, wp) in enumerate(mms):
                dg = dgh_t if wp == "h" else dgl_t
                nc.tensor.matmul(
                    pb[:, t_, pc[0]:pc[1]], dg[:, j, :],
                    etiles[erow][:, hl, mv[0]:mv[1]],
                    start=(i == 0), stop=(i == len(mms) - 1))
            copy_out(bt[:, bi, 0:_OW:2], pb[:, 0, :])
            copy_out(bt[:, bi, 1:_OW - 1:2], pb[:, 1, 0:255])
        nc.gpsimd.dma_start(out=out[:, 254:257, :], in_=bt[:, :, :])

        # ---- main body: 16 pairs of planes x 2 row-chunks
        # stationary groups: 0 = s0/chunkA (rows 0..126), 1 = s0/chunkB, 2 = s1
        for pair in range(_PL // 2):
            g0 = 2 * pair
            for chunk in (0, 1):
                r0 = 0 if chunk == 0 else 128
                t = tpool.tile([128, 2, 2, 256], BF, tag="tin")
                for pg in (0, 1):
                    nc.sync.dma_start(
                        out=t[:, pg, :, :].rearrange("p a w -> p (a w)"),
                        in_=xp[g0 + pg, r0:r0 + 128, :, :].rearrange("r a w -> r (a w)"))

                ig0 = 0 if chunk == 0 else 1
                # psums: [s][t_] with 2 planes packed along the free dim
                ps = {}
                for s, ig in ((0, ig0), (1, 2)):
                    for t_, kxmv in ((0, ((2, (0, 256), (0, 256)), (0, (0, 255), (1, 256)))),
                                     (1, ((1, (0, 255), (0, 255)), (3, (1, 256), (0, 255))))):
                        pt = ppool.tile([128, 2, 256], F32, tag="ps")
                        ps[(s, t_)] = pt
                        mms = []
                        for kx, mv, pc in kxmv:
                            mms.append((ig * 4 + kx, 0, mv, pc, "h"))
                            mms.append((ig * 4 + kx, 1, mv, pc, "h"))
                            if wlo_nz:
                                mms.append((ig * 4 + kx, 0, mv, pc, "l"))
                        for i, (j, hl, mv, pc, wp) in enumerate(mms):
                            st_ = sth_t if wp == "h" else stl_t
                            nc.tensor.matmul(
                                pt[:, :, pc[0]:pc[1]], st_[:, j, :],
                                t[:, :, hl, mv[0]:mv[1]],
                                start=(i == 0), stop=(i == len(mms) - 1))

                # row-pair assembly: chunk A -> oy 0..253, chunk B -> oy 257..510
                # A: [i,0]=s0A[i] (oy 2i),    [i,1]=s1A[i] (oy 2i+1)
                # B: [i,0]=s1B[i] (oy 257+2i), [i,1]=s0B[i] (oy 258+2i)
                rows = ((0, 1) if chunk == 0 else (1, 0))  # s feeding (slot0, slot1)
                for pg in (0, 1):
                    ot = opool.tile([128, 2, _OW], F32, tag="oasm")
                    for slot, s in enumerate(rows):
                        copy_out(ot[0:127, slot, 0:_OW:2], ps[(s, 0)][0:127, pg, :])
                        copy_out(ot[0:127, slot, 1:_OW - 1:2], ps[(s, 1)][0:127, pg, 0:255])
                    dst = out[g0 + pg, 0:254, :] if chunk == 0 else out[g0 + pg, 257:511, :]
                    dst3 = dst.rearrange("(p two) w -> p two w", two=2)
                    # split by partition range: halves the single-engine store
                    # latency (each plain DMA runs on ONE SDMA engine) while
                    # keeping 8KB merged packets -> more DMAs in flight
                    for p0, p1 in ((0, 32), (32, 64), (64, 96), (96, 127)):
                        nc.gpsimd.dma_start(out=dst3[p0:p1], in_=ot[p0:p1, :, :])

    nc.compile()
    return nc


def _host_arrays(w):
    w = np.asarray(w, np.float32)
    w_hi = w.astype(_BF).astype(np.float32)
    w_lo = w - w_hi
    wlo_nz = bool(np.any(w_lo != 0))

    def build_st(wv):
        st = np.zeros((3, 4, 128, 128), np.float32)
        i6 = np.arange(126)
        i7 = np.arange(127)
        for kx in range(4):
            st[0, kx][i6, i6 + 1] = wv[0, kx]        # s0A subdiag, out rows 1..126
            st[0, kx][i7, i7] += wv[2, kx]           # s0A diag, out rows 0..126
            st[1, kx][i7, i7] = wv[0, kx]            # s0B diag
            st[1, kx][i7 + 1, i7] = wv[2, kx]        # s0B sub
            st[2, kx][i7, i7] = wv[1, kx]            # s1 diag
            st[2, kx][i7 + 1, i7] = wv[3, kx]        # s1 sub
        # [g,kx,p,i] -> [p, g*4+kx, i]
        return np.ascontiguousarray(
            st.reshape(12, 128, 128).transpose(1, 0, 2)).astype(_BF)

    def build_dg(wv):
        dg = np.zeros((4, 4, 32, 32), np.float32)
        i = np.arange(32)
        for ky in range(4):
            for kx in range(4):
                dg[ky, kx][i, i] = wv[ky, kx]
        return np.ascontiguousarray(
            dg.reshape(16, 32, 32).transpose(1, 0, 2)).astype(_BF)

    arrs = {"sth": build_st(w_hi), "dgh": build_dg(w_hi)}
    if wlo_nz:
        wlo_b = w_lo.astype(_BF).astype(np.float32)
        arrs["stl"] = build_st(wlo_b)
        arrs["dgl"] = build_dg(wlo_b)
    return wlo_nz, arrs


def kernel(x, kernel):
    global last_exec_ns, last_results
    from concourse.bass_utils import run_bass_kernel_spmd

    x = np.ascontiguousarray(np.asarray(x, np.float32))
    w = np.asarray(kernel, np.float32)[::-1, ::-1]
    wlo_nz, warrs = _host_arrays(w)

    if wlo_nz not in _cache:
        _cache[wlo_nz] = _build(wlo_nz)
    nc = _cache[wlo_nz]

    planes = x.reshape(_NCORES * _PL, _H, _W)
    xpk = np.empty((_NCORES * _PL, _H, 2, _W), dtype=_BF)
    hi = planes.astype(_BF)
    xpk[:, :, 0, :] = hi
    xpk[:, :, 1, :] = (planes - hi.astype(np.float32)).astype(_BF)

    in_maps = []
    for c in range(_NCORES):
        m = {"xp": xpk[c * _PL:(c + 1) * _PL]}
        m.update(warrs)
        in_maps.append(m)

    trace = bool(os.environ.get("BLUR_TRACE"))
    tmpdir = os.environ.get("BLUR_TRACE_DIR") or None
    if trace:
        try:
            res = run_bass_kernel_spmd(nc, in_maps, list(range(_NCORES)),
                                       trace=True, tmpdir=tmpdir)
            last_exec_ns = res.exec_time_ns
        except Exception as e:
            print(f"trace run failed ({type(e).__name__}: {e}); retrying untraced")
            res = run_bass_kernel_spmd(nc, in_maps, list(range(_NCORES)))
            last_exec_ns = None
    else:
        res = run_bass_kernel_spmd(nc, in_maps, list(range(_NCORES)))
        last_exec_ns = None
    last_results = res

    outs = np.stack([res.results[c]["out"] for c in range(_NCORES)])
    return outs.reshape(4, 64, _OW, _OW).astype(np.float32, copy=False)


# revision 10
# speedup vs baseline: 1.2881x; 1.2881x over previous
"""StyleGAN2 up-2x blur (upfirdn2d, up=2, pad=(2,1), 4x4 kernel) on 8 trn2 cores.

x: (4, 64, 256, 256) f32, kernel: (4, 4) f32 -> out: (4, 64, 511, 511) f32.

Polyphase decomposition: out[2r+s, 2c+t] is a 2x2-tap conv of x with weights
from the flipped kernel w = kernel[::-1, ::-1]:
  s=0 -> vertical taps (w[0,kx] @ r-1, w[2,kx] @ r); s=1 -> (w[1,kx] @ r, w[3,kx] @ r+1)
  t=0 -> horizontal taps kx in {0 (c-1), 2 (c)};    t=1 -> kx in {1 (c), 3 (c+1)}

Sharding: pure data parallel over the 256 (N*C) planes, 32 planes/core.

Device algorithm (per core): the vertical 2-tap combine runs on TensorE as
banded-matrix matmuls (stationary [128,128] bands), with the horizontal taps
folded in as column-shifted moving operands accumulating into the same PSUM
bank.  fp32 inputs are split host-side into bf16 hi+lo (x = hi + lo, packed
into one [P,256,2,256] array so loads move 1KB-contiguous chunks); each
logical tap is 2 bf16 matmuls (1 cy/row) with fp32 PSUM accumulation; total
error ~2^-18 rel.  Two planes are packed per matmul (moving free = 512).

Output rows are assembled as row-PAIRS per partition ([127, 2, 511] tiles:
partition i holds rows 2i/2i+1 of a 254-row block), so every store DMA is one
fully contiguous 519KB HBM range with 4088B per-partition chunks.  Rows
254/255/256 (the chunk seam) are computed separately via diagonal matmuls
over plane-partitioned single-row tiles.  Stores go through SWDGE
(nc.gpsimd) which spreads packets over all 16 SDMA engines; HWDGE stores
were measured 10x slower.
"""

import os
import numpy as np
import ml_dtypes

_BF = ml_dtypes.bfloat16
_NCORES = 8
_PL = 32            # planes per core
_H = 256
_W = 256
_OW = 511

_cache = {}
last_exec_ns = None
last_results = None


def _build(wlo_nz: bool):
    from contextlib import ExitStack
    import concourse.mybir as mybir
    import concourse.tile as tile
    from concourse import bacc

    BF = mybir.dt.bfloat16
    F32 = mybir.dt.float32

    nc = bacc.Bacc("TRN2", target_bir_lowering=False, debug=False)
    xp = nc.dram_tensor("xp", [_PL, _H, 2, _W], BF, kind="ExternalInput").ap()
    sth = nc.dram_tensor("sth", [128, 12, 128], BF, kind="ExternalInput").ap()
    dgh = nc.dram_tensor("dgh", [32, 16, 32], BF, kind="ExternalInput").ap()
    if wlo_nz:
        stl = nc.dram_tensor("stl", [128, 12, 128], BF, kind="ExternalInput").ap()
        dgl = nc.dram_tensor("dgl", [32, 16, 32], BF, kind="ExternalInput").ap()
    out = nc.dram_tensor("out", [_PL, _OW, _OW], F32, kind="ExternalOutput").ap()

    ncopy = 0  # alternate evacuation copies between VectorE and ScalarE

    with tile.TileContext(nc) as tc, ExitStack() as ctx:
        cpool = ctx.enter_context(tc.tile_pool(name="const", bufs=1))
        tpool = ctx.enter_context(tc.tile_pool(name="tin", bufs=8))
        epool = ctx.enter_context(tc.tile_pool(name="edge", bufs=1))
        opool = ctx.enter_context(tc.tile_pool(name="oasm", bufs=18))
        bpool = ctx.enter_context(tc.tile_pool(name="bnd", bufs=1))
        ppool = ctx.enter_context(tc.tile_pool(name="ps", bufs=8, space="PSUM"))

        sth_t = cpool.tile([128, 12, 128], BF)
        nc.sync.dma_start(out=sth_t[:, :, :], in_=sth)
        dgh_t = cpool.tile([32, 16, 32], BF)
        nc.sync.dma_start(out=dgh_t[:, :, :], in_=dgh)
        if wlo_nz:
            stl_t = cpool.tile([128, 12, 128], BF)
            nc.sync.dma_start(out=stl_t[:, :, :], in_=stl)
            dgl_t = cpool.tile([32, 16, 32], BF)
            nc.sync.dma_start(out=dgl_t[:, :, :], in_=dgl)

        def copy_out(dst, src):
            nonlocal ncopy
            if ncopy % 2 == 0:
                nc.vector.tensor_copy(out=dst, in_=src)
            else:
                nc.scalar.copy(out=dst, in_=src)
            ncopy += 1

        # ---- seam rows oy=254 (s0,r=127: x[126],x[127]), oy=255 (s1,r=127:
        # ---- x[127],x[128]), oy=256 (s0,r=128: x[127],x[128])
        etiles = {}
        for row in (126, 127, 128):
            e = epool.tile([32, 2, 256], BF, tag=f"e{row}")
            nc.sync.dma_start(out=e[:, :, :].rearrange("g a w -> g (a w)"),
                              in_=xp[:, row, :, :].rearrange("g a w -> g (a w)"))
            etiles[row] = e

        bt = bpool.tile([32, 3, _OW], F32)
        seams = ((0, ((0, 126), (2, 127))),   # oy254: ky=0 on x126, ky=2 on x127
                 (1, ((1, 127), (3, 128))),   # oy255
                 (2, ((0, 127), (2, 128))))   # oy256
        for bi, taps in seams:
            pb = ppool.tile([32, 2, 256], F32, tag="ps")
            mms = []
            # (t_, kx, moving col slice, psum col slice)
            for t_, kx, mv, pc in ((0, 2, (0, 256), (0, 256)), (0, 0, (0, 255), (1, 256)),
                                   (1, 1, (0, 255), (0, 255)), (1, 3, (1, 256), (0, 255))):
                for ky, erow in taps:
                    mms.append((t_, ky * 4 + kx, erow, 0, mv, pc, "h"))
                    mms.append((t_, ky * 4 + kx, erow, 1, mv, pc, "h"))
                    if wlo_nz:
                        mms.append((t_, ky * 4 + kx, erow, 0, mv, pc, "l"))
            for i, (t_, j, erow, hl, mv, pc, wp) in enumerate(mms):
                dg = dgh_t if wp == "h" else dgl_t
                nc.tensor.matmul(
                    pb[:, t_, pc[0]:pc[1]], dg[:, j, :],
                    etiles[erow][:, hl, mv[0]:mv[1]],
                    start=(i == 0), stop=(i == len(mms) - 1))
            copy_out(bt[:, bi, 0:_OW:2], pb[:, 0, :])
            copy_out(bt[:, bi, 1:_OW - 1:2], pb[:, 1, 0:255])
        nc.gpsimd.dma_start(out=out[:, 254:257, :], in_=bt[:, :, :])

        # ---- main body: 16 pairs of planes x 2 row-chunks
        # stationary groups: 0 = s0/chunkA (rows 0..126), 1 = s0/chunkB, 2 = s1
        for pair in range(_PL // 2):
            g0 = 2 * pair
            for chunk in (0, 1):
                r0 = 0 if chunk == 0 else 128
                t = tpool.tile([128, 2, 2, 256], BF, tag="tin")
                for pg in (0, 1):
                    nc.sync.dma_start(
                        out=t[:, pg, :, :].rearrange("p a w -> p (a w)"),
                        in_=xp[g0 + pg, r0:r0 + 128, :, :].rearrange("r a w -> r (a w)"))

                ig0 = 0 if chunk == 0 else 1
                # psums: [s][t_] with 2 planes packed along the free dim
                ps = {}
                for s, ig in ((0, ig0), (1, 2)):
                    for t_, kxmv in ((0, ((2, (0, 256), (0, 256)), (0, (0, 255), (1, 256)))),
                                     (1, ((1, (0, 255), (0, 255)), (3, (1, 256), (0, 255))))):
                        pt = ppool.tile([128, 2, 256], F32, tag="ps")
                        ps[(s, t_)] = pt
                        mms = []
                        for kx, mv, pc in kxmv:
                            mms.append((ig * 4 + kx, 0, mv, pc, "h"))
                            mms.append((ig * 4 + kx, 1, mv, pc, "h"))
                            if wlo_nz:
                                mms.append((ig * 4 + kx, 0, mv, pc, "l"))
                        for i, (j, hl, mv, pc, wp) in enumerate(mms):
                            st_ = sth_t if wp == "h" else stl_t
                            nc.tensor.matmul(
                                pt[:, :, pc[0]:pc[1]], st_[:, j, :],
                                t[:, :, hl, mv[0]:mv[1]],
                                start=(i == 0), stop=(i == len(mms) - 1))

                # row-pair assembly: chunk A -> oy 0..253, chunk B -> oy 257..510
                # A: [i,0]=s0A[i] (oy 2i),    [i,1]=s1A[i] (oy 2i+1)
                # B: [i,0]=s1B[i] (oy 257+2i), [i,1]=s0B[i] (oy 258+2i)
                rows = ((0, 1) if chunk == 0 else (1, 0))  # s feeding (slot0, slot1)
                for pg in (0, 1):
                    ot = opool.tile([128, 2, _OW], F32, tag="oasm")
                    for slot, s in enumerate(rows):
                        copy_out(ot[0:127, slot, 0:_OW:2], ps[(s, 0)][0:127, pg, :])
                        copy_out(ot[0:127, slot, 1:_OW - 1:2], ps[(s, 1)][0:127, pg, 0:255])
                    dst = out[g0 + pg, 0:254, :] if chunk == 0 else out[g0 + pg, 257:511, :]
                    dst3 = dst.rearrange("(p two) w -> p two w", two=2)
                    # split by partition range: halves the single-engine store
                    # latency (each plain DMA runs on ONE SDMA engine) while
                    # keeping 8KB merged packets -> more DMAs in flight
                    nc.gpsimd.dma_start(out=dst3[0:64], in_=ot[0:64, :, :])
                    nc.gpsimd.dma_start(out=dst3[64:127], in_=ot[64:127, :, :])

    nc.compile()
    return nc


def _host_arrays(w):
    w = np.asarray(w, np.float32)
    w_hi = w.astype(_BF).astype(np.float32)
    w_lo = w - w_hi
    wlo_nz = bool(np.any(w_lo != 0))

    def build_st(wv):
        st = np.zeros((3, 4, 128, 128), np.float32)
        i6 = np.arange(126)
        i7 = np.arange(127)
        for kx in range(4):
            st[0, kx][i6, i6 + 1] = wv[0, kx]        # s0A subdiag, out rows 1..126
            st[0, kx][i7, i7] += wv[2, kx]           # s0A diag, out rows 0..126
            st[1, kx][i7, i7] = wv[0, kx]            # s0B diag
            st[1, kx][i7 + 1, i7] = wv[2, kx]        # s0B sub
            st[2, kx][i7, i7] = wv[1, kx]            # s1 diag
            st[2, kx][i7 + 1, i7] = wv[3, kx]        # s1 sub
        # [g,kx,p,i] -> [p, g*4+kx, i]
        return np.ascontiguousarray(
            st.reshape(12, 128, 128).transpose(1, 0, 2)).astype(_BF)

    def build_dg(wv):
        dg = np.zeros((4, 4, 32, 32), np.float32)
        i = np.arange(32)
        for ky in range(4):
            for kx in range(4):
                dg[ky, kx][i, i] = wv[ky, kx]
        return np.ascontiguousarray(
            dg.reshape(16, 32, 32).transpose(1, 0, 2)).astype(_BF)

    arrs = {"sth": build_st(w_hi), "dgh": build_dg(w_hi)}
    if wlo_nz:
        wlo_b = w_lo.astype(_BF).astype(np.float32)
        arrs["stl"] = build_st(wlo_b)
        arrs["dgl"] = build_dg(wlo_b)
    return wlo_nz, arrs


def kernel(x, kernel):
    global last_exec_ns, last_results
    from concourse.bass_utils import run_bass_kernel_spmd

    x = np.ascontiguousarray(np.asarray(x, np.float32))
    w = np.asarray(kernel, np.float32)[::-1, ::-1]
    wlo_nz, warrs = _host_arrays(w)

    if wlo_nz not in _cache:
        _cache[wlo_nz] = _build(wlo_nz)
    nc = _cache[wlo_nz]

    planes = x.reshape(_NCORES * _PL, _H, _W)
    xpk = np.empty((_NCORES * _PL, _H, 2, _W), dtype=_BF)
    hi = planes.astype(_BF)
    xpk[:, :, 0, :] = hi
    xpk[:, :, 1, :] = (planes - hi.astype(np.float32)).astype(_BF)

    in_maps = []
    for c in range(_NCORES):
        m = {"xp": xpk[c * _PL:(c + 1) * _PL]}
        m.update(warrs)
        in_maps.append(m)

    trace = bool(os.environ.get("BLUR_TRACE"))
    tmpdir = os.environ.get("BLUR_TRACE_DIR") or None
    if trace:
        try:
            res = run_bass_kernel_spmd(nc, in_maps, list(range(_NCORES)),
                                       trace=True, tmpdir=tmpdir)
            last_exec_ns = res.exec_time_ns
        except Exception as e:
            print(f"trace run failed ({type(e).__name__}: {e}); retrying untraced")
            res = run_bass_kernel_spmd(nc, in_maps, list(range(_NCORES)))
            last_exec_ns = None
    else:
        res = run_bass_kernel_spmd(nc, in_maps, list(range(_NCORES)))
        last_exec_ns = None
    last_results = res

    outs = np.stack([res.results[c]["out"] for c in range(_NCORES)])
    return outs.reshape(4, 64, _OW, _OW).astype(np.float32, copy=False)


# revision 13
# speedup vs baseline: 1.3601x; 1.0559x over previous
"""StyleGAN2 up-2x blur (upfirdn2d, up=2, pad=(2,1), 4x4 kernel) on 8 trn2 cores.

x: (4, 64, 256, 256) f32, kernel: (4, 4) f32 -> out: (4, 64, 511, 511) f32.

Polyphase decomposition: out[2r+s, 2c+t] is a 2x2-tap conv of x with weights
from the flipped kernel w = kernel[::-1, ::-1]:
  s=0 -> vertical taps (w[0,kx] @ r-1, w[2,kx] @ r); s=1 -> (w[1,kx] @ r, w[3,kx] @ r+1)
  t=0 -> horizontal taps kx in {0 (c-1), 2 (c)};    t=1 -> kx in {1 (c), 3 (c+1)}

Sharding: pure data parallel over the 256 (N*C) planes, 32 planes/core.

Device algorithm (per core): the vertical 2-tap combine runs on TensorE as
banded-matrix matmuls (stationary [128,128] bands), with the horizontal taps
folded in as column-shifted moving operands accumulating into the same PSUM
bank.  fp32 inputs are split host-side into bf16 hi+lo (x = hi + lo, packed
into one [P,256,2,256] array so loads move 1KB-contiguous chunks); each
logical tap is 2 bf16 matmuls (1 cy/row) with fp32 PSUM accumulation; total
error ~2^-18 rel.  Two planes are packed per matmul (moving free = 512).

Output rows are assembled as row-PAIRS per partition ([127, 2, 511] tiles:
partition i holds rows 2i/2i+1 of a 254-row block), so every store DMA is one
fully contiguous 519KB HBM range with 4088B per-partition chunks.  Rows
254/255/256 (the chunk seam) are computed separately via diagonal matmuls
over plane-partitioned single-row tiles.  Stores go through SWDGE
(nc.gpsimd) which spreads packets over all 16 SDMA engines; HWDGE stores
were measured 10x slower.
"""

import os
import numpy as np
import ml_dtypes

_BF = ml_dtypes.bfloat16
_NCORES = 8
_PL = 32            # planes per core
_H = 256
_W = 256
_OW = 511

_cache = {}
last_exec_ns = None
last_results = None


def _build(wlo_nz: bool):
    from contextlib import ExitStack
    import concourse.mybir as mybir
    import concourse.tile as tile
    from concourse import bacc

    BF = mybir.dt.bfloat16
    F32 = mybir.dt.float32

    nc = bacc.Bacc("TRN2", target_bir_lowering=False, debug=False)
    xp = nc.dram_tensor("xp", [_PL, _H, 2, _W], BF, kind="ExternalInput").ap()
    sth = nc.dram_tensor("sth", [128, 12, 128], BF, kind="ExternalInput").ap()
    dgh = nc.dram_tensor("dgh", [32, 16, 32], BF, kind="ExternalInput").ap()
    if wlo_nz:
        stl = nc.dram_tensor("stl", [128, 12, 128], BF, kind="ExternalInput").ap()
        dgl = nc.dram_tensor("dgl", [32, 16, 32], BF, kind="ExternalInput").ap()
    out = nc.dram_tensor("out", [_PL, _OW, _OW], F32, kind="ExternalOutput").ap()

    ncopy = 0  # alternate evacuation copies between VectorE and ScalarE

    with tile.TileContext(nc) as tc, ExitStack() as ctx:
        cpool = ctx.enter_context(tc.tile_pool(name="const", bufs=1))
        tpool = ctx.enter_context(tc.tile_pool(name="tin", bufs=8))
        epool = ctx.enter_context(tc.tile_pool(name="edge", bufs=1))
        opool = ctx.enter_context(tc.tile_pool(name="oasm", bufs=18))
        bpool = ctx.enter_context(tc.tile_pool(name="bnd", bufs=1))
        ppool = ctx.enter_context(tc.tile_pool(name="ps", bufs=8, space="PSUM"))

        sth_t = cpool.tile([128, 12, 128], BF)
        nc.sync.dma_start(out=sth_t[:, :, :], in_=sth)
        dgh_t = cpool.tile([32, 16, 32], BF)
        nc.sync.dma_start(out=dgh_t[:, :, :], in_=dgh)
        if wlo_nz:
            stl_t = cpool.tile([128, 12, 128], BF)
            nc.sync.dma_start(out=stl_t[:, :, :], in_=stl)
            dgl_t = cpool.tile([32, 16, 32], BF)
            nc.sync.dma_start(out=dgl_t[:, :, :], in_=dgl)

        def copy_out(dst, src):
            nonlocal ncopy
            if ncopy % 2 == 0:
                nc.vector.tensor_copy(out=dst, in_=src)
            else:
                nc.scalar.copy(out=dst, in_=src)
            ncopy += 1

        # ---- seam rows oy=254 (s0,r=127: x[126],x[127]), oy=255 (s1,r=127:
        # ---- x[127],x[128]), oy=256 (s0,r=128: x[127],x[128])
        etiles = {}
        for row in (126, 127, 128):
            e = epool.tile([32, 2, 256], BF, tag=f"e{row}")
            nc.sync.dma_start(out=e[:, :, :].rearrange("g a w -> g (a w)"),
                              in_=xp[:, row, :, :].rearrange("g a w -> g (a w)"))
            etiles[row] = e

        bt = bpool.tile([32, 3, _OW], F32)
        seams = ((0, ((0, 126), (2, 127))),   # oy254: ky=0 on x126, ky=2 on x127
                 (1, ((1, 127), (3, 128))),   # oy255
                 (2, ((0, 127), (2, 128))))   # oy256
        for bi, taps in seams:
            pb = ppool.tile([32, 2, 256], F32, tag="ps")
            mms = []
            # (t_, kx, moving col slice, psum col slice)
            for t_, kx, mv, pc in ((0, 2, (0, 256), (0, 256)), (0, 0, (0, 255), (1, 256)),
                                   (1, 1, (0, 255), (0, 255)), (1, 3, (1, 256), (0, 255))):
                for ky, erow in taps:
                    mms.append((t_, ky * 4 + kx, erow, 0, mv, pc, "h"))
                    mms.append((t_, ky * 4 + kx, erow, 1, mv, pc, "h"))
                    if wlo_nz:
                        mms.append((t_, ky * 4 + kx, erow, 0, mv, pc, "l"))
            for i, (t_, j, erow, hl, mv, pc, wp) in enumerate(mms):
                dg = dgh_t if wp == "h" else dgl_t
                nc.tensor.matmul(
                    pb[:, t_, pc[0]:pc[1]], dg[:, j, :],
                    etiles[erow][:, hl, mv[0]:mv[1]],
                    start=(i == 0), stop=(i == len(mms) - 1))
            copy_out(bt[:, bi, 0:_OW:2], pb[:, 0, :])
            copy_out(bt[:, bi, 1:_OW - 1:2], pb[:, 1, 0:255])
        nc.gpsimd.dma_start(out=out[:, 254:257, :], in_=bt[:, :, :])

        # ---- main body: 16 pairs of planes x 2 row-chunks
        # stationary groups: 0 = s0/chunkA (rows 0..126), 1 = s0/chunkB, 2 = s1
        for pair in range(_PL // 2):
            g0 = 2 * pair
            for chunk in (0, 1):
                r0 = 0 if chunk == 0 else 128
                t = tpool.tile([128, 2, 2, 256], BF, tag="tin")
                for pg in (0, 1):
                    nc.sync.dma_start(
                        out=t[:, pg, :, :].rearrange("p a w -> p (a w)"),
                        in_=xp[g0 + pg, r0:r0 + 128, :, :].rearrange("r a w -> r (a w)"))

                ig0 = 0 if chunk == 0 else 1
                # row-pair assembly tiles allocated up-front so each psum
                # group's copies can be emitted right after its matmuls
                # (drains psum banks earlier -> denser PE stream)
                rows = ((0, 1) if chunk == 0 else (1, 0))  # s feeding (slot0, slot1)
                ot0 = opool.tile([128, 2, _OW], F32, tag="oasm")
                ot1 = opool.tile([128, 2, _OW], F32, tag="oasm")
                ots = (ot0, ot1)
                # psums: [s][t_] with 2 planes packed along the free dim
                ps = {}
                for s, ig in ((0, ig0), (1, 2)):
                    for t_, kxmv in ((0, ((2, (0, 256), (0, 256)), (0, (0, 255), (1, 256)))),
                                     (1, ((1, (0, 255), (0, 255)), (3, (1, 256), (0, 255))))):
                        pt = ppool.tile([128, 2, 256], F32, tag="ps")
                        ps[(s, t_)] = pt
                        mms = []
                        for kx, mv, pc in kxmv:
                            mms.append((ig * 4 + kx, 0, mv, pc, "h"))
                            mms.append((ig * 4 + kx, 1, mv, pc, "h"))
                            if wlo_nz:
                                mms.append((ig * 4 + kx, 0, mv, pc, "l"))
                        for i, (j, hl, mv, pc, wp) in enumerate(mms):
                            st_ = sth_t if wp == "h" else stl_t
                            nc.tensor.matmul(
                                pt[:, :, pc[0]:pc[1]], st_[:, j, :],
                                t[:, :, hl, mv[0]:mv[1]],
                                start=(i == 0), stop=(i == len(mms) - 1))
                        # drain this psum group immediately (overlaps with the
                        # next group's matmuls); chunk A row-pair layout:
                        # [i,0]=s0A[i] (oy 2i), [i,1]=s1A[i] (oy 2i+1);
                        # chunk B: [i,0]=s1B[i] (oy 257+2i), [i,1]=s0B[i]
                        slot = rows.index(s)
                        for pg in (0, 1):
                            if t_ == 0:
                                copy_out(ots[pg][0:127, slot, 0:_OW:2],
                                         pt[0:127, pg, :])
                            else:
                                copy_out(ots[pg][0:127, slot, 1:_OW - 1:2],
                                         pt[0:127, pg, 0:255])

                for pg in (0, 1):
                    ot = ots[pg]
                    dst = out[g0 + pg, 0:254, :] if chunk == 0 else out[g0 + pg, 257:511, :]
                    dst3 = dst.rearrange("(p two) w -> p two w", two=2)
                    # split by partition range: halves the single-engine store
                    # latency (each plain DMA runs on ONE SDMA engine) while
                    # keeping 8KB merged packets -> more DMAs in flight
                    nc.gpsimd.dma_start(out=dst3[0:64], in_=ot[0:64, :, :])
                    nc.gpsimd.dma_start(out=dst3[64:127], in_=ot[64:127, :, :])

    nc.compile()
    return nc


def _host_arrays(w):
    w = np.asarray(w, np.float32)
    w_hi = w.astype(_BF).astype(np.float32)
    w_lo = w - w_hi
    wlo_nz = bool(np.any(w_lo != 0))

    def build_st(wv):
        st = np.zeros((3, 4, 128, 128), np.float32)
        i6 = np.arange(126)
        i7 = np.arange(127)
        for kx in range(4):
            st[0, kx][i6, i6 + 1] = wv[0, kx]        # s0A subdiag, out rows 1..126
            st[0, kx][i7, i7] += wv[2, kx]           # s0A diag, out rows 0..126
            st[1, kx][i7, i7] = wv[0, kx]            # s0B diag
            st[1, kx][i7 + 1, i7] = wv[2, kx]        # s0B sub
            st[2, kx][i7, i7] = wv[1, kx]            # s1 diag
            st[2, kx][i7 + 1, i7] = wv[3, kx]        # s1 sub
        # [g,kx,p,i] -> [p, g*4+kx, i]
        return np.ascontiguousarray(
            st.reshape(12, 128, 128).transpose(1, 0, 2)).astype(_BF)

    def build_dg(wv):
        dg = np.zeros((4, 4, 32, 32), np.float32)
        i = np.arange(32)
        for ky in range(4):
            for kx in range(4):
                dg[ky, kx][i, i] = wv[ky, kx]
        return np.ascontiguousarray(
            dg.reshape(16, 32, 32).transpose(1, 0, 2)).astype(_BF)

    arrs = {"sth": build_st(w_hi), "dgh": build_dg(w_hi)}
    if wlo_nz:
        wlo_b = w_lo.astype(_BF).astype(np.float32)
        arrs["stl"] = build_st(wlo_b)
        arrs["dgl"] = build_dg(wlo_b)
    return wlo_nz, arrs


def kernel(x, kernel):
    global last_exec_ns, last_results
    from concourse.bass_utils import run_bass_kernel_spmd

    x = np.ascontiguousarray(np.asarray(x, np.float32))
    w = np.asarray(kernel, np.float32)[::-1, ::-1]
    wlo_nz, warrs = _host_arrays(w)

    if wlo_nz not in _cache:
        _cache[wlo_nz] = _build(wlo_nz)
    nc = _cache[wlo_nz]

    planes = x.reshape(_NCORES * _PL, _H, _W)
    xpk = np.empty((_NCORES * _PL, _H, 2, _W), dtype=_BF)
    hi = planes.astype(_BF)
    xpk[:, :, 0, :] = hi
    xpk[:, :, 1, :] = (planes - hi.astype(np.float32)).astype(_BF)

    in_maps = []
    for c in range(_NCORES):
        m = {"xp": xpk[c * _PL:(c + 1) * _PL]}
        m.update(warrs)
        in_maps.append(m)

    trace = bool(os.environ.get("BLUR_TRACE"))
    tmpdir = os.environ.get("BLUR_TRACE_DIR") or None
    if trace:
        try:
            res = run_bass_kernel_spmd(nc, in_maps, list(range(_NCORES)),
                                       trace=True, tmpdir=tmpdir)
            last_exec_ns = res.exec_time_ns
        except Exception as e:
            print(f"trace run failed ({type(e).__name__}: {e}); retrying untraced")
            res = run_bass_kernel_spmd(nc, in_maps, list(range(_NCORES)))
            last_exec_ns = None
    else:
        res = run_bass_kernel_spmd(nc, in_maps, list(range(_NCORES)))
        last_exec_ns = None
    last_results = res

    outs = np.stack([res.results[c]["out"] for c in range(_NCORES)])
    return outs.reshape(4, 64, _OW, _OW).astype(np.float32, copy=False)
